# revision 11
# baseline (speedup 1.0000x reference)
"""MoE layer (E=8 experts, top-2 routing) on 8 TRN2 NeuronCores.

Strategy (expert-parallel, matching the sharding hint):
  - Host computes the (tiny) router: logits -> top-2 experts + combine
    weights per token, then dispatches: gathers each expert's tokens into
    a padded [Cpad, D] buffer per core (the "all-to-all dispatch" done as
    part of host-side sharding).
  - Core e runs expert e's FFN over its tokens:
        h = gelu(x @ W1[e].T + b1[e]);  y = (h @ W2[e].T) * gate
    as tiled TensorE matmuls (bf16 operands, fp32 PSUM accumulation).
    Tokens stay resident in SBUF; each weight byte is streamed from HBM
    exactly once.
  - Host combines: out[token] += y rows (the "all-to-all combine").

Self-contained: hardcodes D=1024, F=4096, E=8, K=2; pads the per-expert
token count to a multiple of 256 at runtime and compiles one SPMD NEFF
for all 8 cores.
"""

import sys

import numpy as np

for _p in ("/opt/trn_rl_repo",):
    if _p not in sys.path:
        sys.path.append(_p)

import ml_dtypes
from contextlib import ExitStack

import concourse.bacc as bacc
import concourse.mybir as mybir
from concourse.tile import TileContext
from concourse.bass_utils import run_bass_kernel_spmd

D = 1024
F = 4096
E = 8
TOP_K = 2
P = 128
DT = D // P   # 8 k-tiles for stage 1
FT = F // P   # 32 f tiles
N_CORES = 8

BF16 = mybir.dt.bfloat16
F32 = mybir.dt.float32
NP_BF16 = ml_dtypes.bfloat16

_nc_cache = {}


def _round_up(v, m):
    return ((v + m - 1) // m) * m


def _chunks(total, size):
    out = []
    o = 0
    while o < total:
        out.append((o, min(size, total - o)))
        o += size
    return out


def build_moe_nc(cpad, loop_n=1):
    """One SPMD Bass program: expert FFN over [cpad, D] gathered tokens.

    loop_n > 1 repeats the compute body on-device (used only for timing:
    amortizes the host->device dispatch overhead across N iterations).
    """
    ct_n = cpad // P
    nc = bacc.Bacc("TRN2", target_bir_lowering=False, debug=False,
                   num_devices=N_CORES)

    xet = nc.dram_tensor("xet", [D, cpad], BF16, kind="ExternalInput")
    # wt1 pre-tiled on host: wt1[f, p, dt*P + m] = W1[e][f*P + m, dt*P + p]
    wt1 = nc.dram_tensor("wt1", [FT, P, DT * P], BF16, kind="ExternalInput")
    wt2 = nc.dram_tensor("wt2", [F, D], BF16, kind="ExternalInput")
    b1t = nc.dram_tensor("b1t", [P, FT], F32, kind="ExternalInput")
    gt = nc.dram_tensor("gt", [P, ct_n], F32, kind="ExternalInput")
    yo = nc.dram_tensor("y", [cpad, D], F32, kind="ExternalOutput")

    with TileContext(nc) as tc, ExitStack() as ctx:
        # Resident loads go on the gpsimd (SWDGE) queue so they never
        # head-of-line-block the per-f weight stream on the sync queue;
        # xet is loaded in column chunks so the first matmuls only wait
        # for the first chunk.
        const = ctx.enter_context(tc.tile_pool(name="const", bufs=1))
        b1_sb = const.tile([P, FT], F32, tag="b1")
        nc.gpsimd.dma_start(out=b1_sb[:], in_=b1t[:])
        gt_sb = const.tile([P, ct_n], F32, tag="gt")
        nc.gpsimd.dma_start(out=gt_sb[:], in_=gt[:])

        xpool = ctx.enter_context(tc.tile_pool(name="xet", bufs=1))
        xet_sb = []
        for dt in range(DT):
            t = xpool.tile([P, cpad], BF16, tag=f"xet{dt}")
            xet_sb.append(t)
        for (c0, cw) in _chunks(cpad, 512):
            for dt in range(DT):
                nc.gpsimd.dma_start(
                    out=xet_sb[dt][:, c0:c0 + cw],
                    in_=xet[dt * P:(dt + 1) * P, c0:c0 + cw])

        w2pool = ctx.enter_context(tc.tile_pool(name="wt2", bufs=1))
        wt2_sb = []
        for f in range(FT):
            t = w2pool.tile([P, D], BF16, tag=f"wt2_{f}")
            nc.gpsimd.dma_start(out=t[:], in_=wt2[f * P:(f + 1) * P, :])
            wt2_sb.append(t)

        hpool = ctx.enter_context(tc.tile_pool(name="h", bufs=1))
        h_all = hpool.tile([P, FT * cpad], BF16, tag="h")

        w1pool = ctx.enter_context(tc.tile_pool(name="wt1", bufs=3))
        ps1pool = ctx.enter_context(tc.tile_pool(name="ps1", bufs=3, space="PSUM"))
        ps2pool = ctx.enter_context(tc.tile_pool(name="ps2", bufs=2, space="PSUM"))
        ypool = ctx.enter_context(tc.tile_pool(name="ys", bufs=3))

        cchunks = _chunks(cpad, 512)

        loop_ctx = (
            tc.For_i(0, loop_n, 1, hint_engines=(mybir.EngineType.PE,))
            if loop_n > 1 else None
        )
        if loop_ctx is not None:
            ctx.enter_context(loop_ctx)

        # Stage 1: hT[f, c] = gelu(sum_d WT1[d, f] * xT[d, c] + b1[f])
        for f in range(FT):
            w1f = w1pool.tile([P, DT * P], BF16, tag="w1f")
            nc.sync.dma_start(out=w1f[:], in_=wt1[f, :, :])
            for (c0, cw) in cchunks:
                ps = ps1pool.tile([P, cw], F32, tag="ps1")
                for dt in range(DT):
                    nc.tensor.matmul(
                        ps[:, :cw],
                        w1f[:, dt * P:(dt + 1) * P],
                        xet_sb[dt][:, c0:c0 + cw],
                        start=(dt == 0),
                        stop=(dt == DT - 1),
                    )
                nc.scalar.activation(
                    h_all[:, f * cpad + c0:f * cpad + c0 + cw],
                    ps[:, :cw],
                    mybir.ActivationFunctionType.Gelu,
                    bias=b1_sb[:, f:f + 1],
                    scale=1.0,
                )

        # Stage 2: y[c, d] = (sum_f hT[f, c] * WT2[f, d]) * gate[c]
        for ct in range(ct_n):
            ys = ypool.tile([P, D], F32, tag="ys")
            for dc in range(D // 512):
                ps2 = ps2pool.tile([P, 512], F32, tag="ps2")
                for f in range(FT):
                    nc.tensor.matmul(
                        ps2[:],
                        h_all[:, f * cpad + ct * P:f * cpad + ct * P + P],
                        wt2_sb[f][:, dc * 512:(dc + 1) * 512],
                        start=(f == 0),
                        stop=(f == FT - 1),
                    )
                nc.vector.tensor_scalar_mul(
                    ys[:, dc * 512:(dc + 1) * 512], ps2[:], gt_sb[:, ct:ct + 1]
                )
            nc.sync.dma_start(out=yo[ct * P:(ct + 1) * P, :], in_=ys[:])

    nc.compile()
    return nc


def _get_nc(cpad, loop_n=1):
    key = (cpad, loop_n)
    if key not in _nc_cache:
        _nc_cache[key] = build_moe_nc(cpad, loop_n)
    return _nc_cache[key]


def _route(xf, Wr):
    """Top-2 routing on host (float64 for a stable argmax order)."""
    logits = xf.astype(np.float64) @ Wr.astype(np.float64).T  # [T, E]
    order = np.argsort(-logits, axis=1, kind="stable")
    top_i = order[:, :TOP_K]                                   # [T, 2]
    top_l = np.take_along_axis(logits, top_i, axis=1)
    m = top_l.max(axis=1, keepdims=True)
    ex = np.exp(top_l - m)
    gate = (ex / ex.sum(axis=1, keepdims=True)).astype(np.float32)  # [T, 2]
    return top_i, gate


def make_in_maps(x, Wr, W1, b1, W2, b2):
    """Host-side shard: routing, per-expert gather, weight transposes."""
    B, S, _ = x.shape
    T = B * S
    xf = np.asarray(x, dtype=np.float32).reshape(T, D)
    top_i, gate = _route(xf, np.asarray(Wr, dtype=np.float32))

    idx_list, gate_list = [], []
    for e in range(E):
        t_idx, k_idx = np.nonzero(top_i == e)
        idx_list.append(t_idx.astype(np.int64))
        gate_list.append(gate[t_idx, k_idx])

    cmax = max(1, max(len(i) for i in idx_list))
    cpad = _round_up(max(cmax, 128), 128)

    xfT = np.ascontiguousarray(xf.T).astype(NP_BF16)  # [D, T] bf16

    in_maps = []
    for e in range(E):
        idx = idx_list[e]
        ce = len(idx)
        xet = np.zeros((D, cpad), dtype=NP_BF16)
        xet[:, :ce] = xfT[:, idx]
        gt = np.zeros(cpad, dtype=np.float32)
        gt[:ce] = gate_list[e]
        in_maps.append({
            "xet": xet,
            "wt1": np.ascontiguousarray(
                np.asarray(W1[e], dtype=np.float32)
                .astype(NP_BF16)
                .reshape(FT, P, DT, P)
                .transpose(0, 3, 2, 1)
            ).reshape(FT, P, DT * P),
            "wt2": np.ascontiguousarray(
                np.asarray(W2[e], dtype=np.float32).T).astype(NP_BF16),
            "b1t": np.ascontiguousarray(
                np.asarray(b1[e], dtype=np.float32).reshape(FT, P).T),
            "gt": np.ascontiguousarray(
                gt.reshape(cpad // P, P).T),
        })
    return in_maps, idx_list, cpad, top_i, gate


def combine(results, idx_list, x_shape, top_i, gate, b2):
    B, S, _ = x_shape
    T = B * S
    out = np.zeros((T, D), dtype=np.float32)
    for e in range(E):
        idx = idx_list[e]
        if len(idx):
            out[idx] += results[e]["y"][:len(idx)]
    b2 = np.asarray(b2, dtype=np.float32)
    if np.any(b2):
        comb = np.zeros((T, E), dtype=np.float32)
        comb[np.arange(T)[:, None], top_i] = gate
        out += comb @ b2
    return out.reshape(B, S, D)


def kernel(x, Wr, W1, b1, W2, b2):
    in_maps, idx_list, cpad, top_i, gate = make_in_maps(x, Wr, W1, b1, W2, b2)
    nc = _get_nc(cpad)
    res = run_bass_kernel_spmd(nc, in_maps, list(range(N_CORES)))
    return combine(res.results, idx_list, x.shape, top_i, gate, b2)
